# revision 6
# baseline (speedup 1.0000x reference)
"""MultiHeadGAT layer on 8 trn2 NeuronCores, data-parallel over batch.

Per core (one batch element):
  Wh = h @ W                                  [1024, 512]  (heads=8, fo=64)
  e_src[h,i] = Wh[i, h*64:+64] @ a_src[h]     e_dst similar
  scores_T[j,i] = leaky_relu(e_src[i] + e_dst[j])   (transposed layout: j on partitions)
  P = exp(scores_T) * adjT   (exp(leaky(s)) == max(exp(s), exp(0.2 s)))
  out[i, h*64+f] = (P.T @ Wh_h)[i,f] / sum_j P[j,i]

The AV matmul runs in the transposed orientation: out_T[f,i] = sum_j Wh[j,f]*P[j,i],
with a ones column appended to the lhsT so row 64 of the accumulator is the
softmax denominator. Final transpose back + scale by reciprocal.
"""
import sys

sys.path.insert(0, "/opt/trn_rl_repo")

import numpy as np

import concourse.bass as bass
import concourse.mybir as mybir
import concourse.tile as tile
from concourse.bass_utils import run_bass_kernel_spmd
from concourse.masks import make_identity

F32 = mybir.dt.float32
I32 = mybir.dt.int32
AF = mybir.ActivationFunctionType

N_CORES = 8
N = 1024
NB = 8          # row blocks of 128
FIN = 256
KT = 2          # FIN / 128
FO = 512        # heads * fo
MT = 4          # FO / 128
H = 8
FOH = 64
ALPHA = 0.2

_MAX_SYNC_WAITS = 1


def _split_sync_waits(nc, max_waits=_MAX_SYNC_WAITS):
    """This walrus build rejects instructions carrying more than one sync
    wait; hoist extras onto NOPs inserted just before, on the same engine."""
    uid = 0
    for f in nc.m.functions:
        for bb in f.blocks:
            out = []
            for inst in bb.instructions:
                si = getattr(inst, "sync_info", None)
                if si is not None and si.on_wait and len(si.on_wait) > max_waits:
                    waits = list(si.on_wait)
                    keep = waits[-max_waits:]
                    extra = waits[:-max_waits]
                    si.on_wait.clear()
                    si.on_wait.extend(keep)
                    while extra:
                        chunk, extra = extra[:max_waits], extra[max_waits:]
                        nop = mybir.InstNoOp(
                            name=f"waitsplit-{uid}",
                            engine=inst.engine,
                            sync_info=mybir.SyncInfo(
                                on_wait=list(chunk), on_update=[]
                            ),
                            bass_nofuse=True,
                        )
                        uid += 1
                        out.append(nop)
                out.append(inst)
            bb.instructions[:] = out


def build_nc(split=True):
    nc = bass.Bass()
    h_d = nc.declare_dram_parameter("h", [N, FIN], F32, isOutput=False)
    adj_d = nc.declare_dram_parameter("adj", [N, N], I32, isOutput=False)
    w_d = nc.declare_dram_parameter("W", [FIN, FO], F32, isOutput=False)
    a_d = nc.declare_dram_parameter("A", [FO, 2 * H], F32, isOutput=False)
    out_d = nc.declare_dram_parameter("out", [N, FO], F32, isOutput=True)

    with tile.TileContext(nc) as tc:
        with (
            tc.tile_pool(name="const", bufs=1) as const,
            tc.tile_pool(name="persist", bufs=1) as persist,
            tc.tile_pool(name="ld", bufs=2) as ld,
            tc.tile_pool(name="x1p", bufs=3) as x1p,
            tc.tile_pool(name="x2p", bufs=2) as x2p,
            tc.tile_pool(name="epi", bufs=2) as epi,
            tc.tile_pool(name="psS", bufs=3, space="PSUM") as psS,
            tc.tile_pool(name="psAcc", bufs=2, space="PSUM") as psAcc,
        ):
            ident = const.tile([128, 128], F32, tag="ident")
            make_identity(nc, ident[:])
            sel = []
            for hh in range(H):
                t = const.tile([16, 128], F32, tag=f"sel{hh}", name=f"sel{hh}")
                nc.gpsimd.memset(t[:], 0.0)
                # t[p, y] = (p == hh) ? 1.0 : 0.0
                nc.gpsimd.affine_select(
                    out=t[:], in_=t[:], pattern=[[0, 128]],
                    compare_op=mybir.AluOpType.not_equal, fill=1.0,
                    base=-hh, channel_multiplier=1,
                )
                sel.append(t)

            wk = []
            for k in range(KT):
                t = const.tile([128, FO], F32, tag=f"W{k}")
                nc.sync.dma_start(t[:], w_d[k * 128:(k + 1) * 128, :])
                wk.append(t)
            ak = []
            for k in range(MT):
                t = const.tile([128, 2 * H], F32, tag=f"A{k}")
                nc.sync.dma_start(t[:], a_d[k * 128:(k + 1) * 128, :])
                ak.append(t)

            # ---- hT[k][f128, i] = h[i, k*128+f] ----
            hT = [persist.tile([128, N], F32, tag=f"hT{k}", name=f"hT{k}") for k in range(KT)]
            for ib in range(NB):
                ht = ld.tile([128, FIN], F32, tag="hld")
                nc.sync.dma_start(ht[:], h_d[ib * 128:(ib + 1) * 128, :])
                for k in range(KT):
                    tp = psS.tile([128, 512], F32, tag="ps")
                    nc.tensor.transpose(
                        tp[:, 0:128], ht[:, k * 128:(k + 1) * 128], ident[:]
                    )
                    nc.vector.tensor_copy(
                        hT[k][:, ib * 128:(ib + 1) * 128], tp[:, 0:128]
                    )

            # ---- Wh_aug[jb][:, hh*65:+64] = (h @ W) block, col hh*65+64 = ones ----
            wh_aug = [persist.tile([128, H * 65], F32, tag=f"wha{j}", name=f"wha{j}") for j in range(NB)]
            for jb in range(NB):
                ps = psS.tile([128, 512], F32, tag="ps")
                for k in range(KT):
                    nc.tensor.matmul(
                        ps[:], hT[k][:, jb * 128:(jb + 1) * 128], wk[k][:],
                        start=(k == 0), stop=(k == KT - 1),
                    )
                for hh in range(H):
                    nc.vector.tensor_copy(
                        wh_aug[jb][:, hh * 65:hh * 65 + 64],
                        ps[:, hh * 64:(hh + 1) * 64],
                    )
                for hh in range(H):
                    nc.gpsimd.memset(wh_aug[jb][:, hh * 65 + 64:hh * 65 + 65], 1.0)

            # ---- WhT[m][f128, i] = Wh[i, m*128+f] ----
            whT = [persist.tile([128, N], F32, tag=f"whT{m}", name=f"whT{m}") for m in range(MT)]
            for m in range(MT):
                for c in range(2):
                    ps = psS.tile([128, 512], F32, tag="ps")
                    for k in range(KT):
                        nc.tensor.matmul(
                            ps[:], wk[k][:, m * 128:(m + 1) * 128],
                            hT[k][:, c * 512:(c + 1) * 512],
                            start=(k == 0), stop=(k == KT - 1),
                        )
                    nc.vector.tensor_copy(whT[m][:, c * 512:(c + 1) * 512], ps[:])

            # ---- E_T[16, i]: rows 0..7 = e_src per head, 8..15 = e_dst ----
            e_t = const.tile([16, N], F32, tag="eT")
            for c in range(2):
                ps = psS.tile([16, 512], F32, tag="ps")
                for m in range(MT):
                    nc.tensor.matmul(
                        ps[:], ak[m][:], whT[m][:, c * 512:(c + 1) * 512],
                        start=(m == 0), stop=(m == MT - 1),
                    )
                nc.vector.tensor_copy(e_t[:, c * 512:(c + 1) * 512], ps[:])

            # ---- E[jb][p, 16] = E_T[:, jb*128+p] (per-partition bias columns) ----
            e_sb = [persist.tile([128, 16], F32, tag=f"E{j}", name=f"E{j}") for j in range(NB)]
            e_sc = [persist.tile([128, 16], F32, tag=f"Es{j}", name=f"Es{j}") for j in range(NB)]
            for jb in range(NB):
                tp = psS.tile([128, 512], F32, tag="ps")
                nc.tensor.transpose(
                    tp[:, 0:16], e_t[:, jb * 128:(jb + 1) * 128], ident[0:16, 0:16]
                )
                nc.vector.tensor_copy(e_sb[jb][:], tp[:, 0:16])
                nc.vector.tensor_scalar_mul(e_sc[jb][:], tp[:, 0:16], ALPHA)

            # ---- e_srcb[h][p, i] = e_src[h, i] broadcast over partitions ----
            e_srcb = [persist.tile([128, N], F32, tag=f"esb{hh}", name=f"esb{hh}") for hh in range(H)]
            for hh in range(H):
                for c in range(2):
                    ps = psS.tile([128, 512], F32, tag="ps")
                    nc.tensor.matmul(
                        ps[:], sel[hh][:], e_t[:, c * 512:(c + 1) * 512],
                        start=True, stop=True,
                    )
                    nc.vector.tensor_copy(e_srcb[hh][:, c * 512:(c + 1) * 512], ps[:])

            # ---- adjT[jb][j128, i] = adj[i, jb*128+j] as f32 ----
            adjT = [persist.tile([128, N], F32, tag=f"adjT{j}", name=f"adjT{j}") for j in range(NB)]
            for ib in range(NB):
                ai = ld.tile([128, N], I32, tag="adji")
                nc.sync.dma_start(ai[:], adj_d[ib * 128:(ib + 1) * 128, :])
                af = ld.tile([128, N], F32, tag="adjf")
                nc.vector.tensor_copy(af[:], ai[:])
                for jb in range(NB):
                    tp = psS.tile([128, 512], F32, tag="ps")
                    nc.tensor.transpose(
                        tp[:, 0:128], af[:, jb * 128:(jb + 1) * 128], ident[:]
                    )
                    nc.vector.tensor_copy(
                        adjT[jb][:, ib * 128:(ib + 1) * 128], tp[:, 0:128]
                    )

            # ---- output staging ----
            out_sb = [persist.tile([128, FO], F32, tag=f"osb{c}", name=f"osb{c}") for c in range(NB)]

            # ---- main attention loop ----
            for hh in range(H):
                acc = [psAcc.tile([65, 512], F32, tag=f"acc{c}", name=f"acc{c}") for c in range(2)]
                for jb in range(NB):
                    x1 = x1p.tile([128, N], F32, tag="x1")
                    nc.scalar.activation(
                        x1[:], e_srcb[hh][:], AF.Exp,
                        bias=e_sb[jb][:, 8 + hh:9 + hh],
                    )
                    x2 = x2p.tile([128, N], F32, tag="x2")
                    nc.scalar.activation(
                        x2[:], e_srcb[hh][:], AF.Exp,
                        bias=e_sc[jb][:, 8 + hh:9 + hh], scale=ALPHA,
                    )
                    nc.vector.tensor_max(x1[:], x1[:], x2[:])
                    nc.vector.tensor_mul(x1[:], x1[:], adjT[jb][:])
                    for c in range(2):
                        nc.tensor.matmul(
                            acc[c][:],
                            wh_aug[jb][:, hh * 65:(hh + 1) * 65],
                            x1[:, c * 512:(c + 1) * 512],
                            start=(jb == 0), stop=(jb == NB - 1),
                        )
                # epilogue: transpose [65, i] chunks back, scale by 1/sum
                acc_sb = epi.tile([65, N], F32, tag="accsb")
                for c in range(2):
                    nc.vector.tensor_copy(acc_sb[:, c * 512:(c + 1) * 512], acc[c][:])
                for c in range(NB):
                    tp = psS.tile([128, 512], F32, tag="ps")
                    nc.tensor.transpose(
                        tp[:, 0:65], acc_sb[:, c * 128:(c + 1) * 128],
                        ident[0:65, 0:65],
                    )
                    rec = epi.tile([128, 1], F32, tag="rec")
                    nc.vector.reciprocal(rec[:], tp[:, 64:65])
                    nc.vector.tensor_scalar_mul(
                        out_sb[c][:, hh * FOH:(hh + 1) * FOH], tp[:, 0:64], rec[:]
                    )

            for c in range(NB):
                nc.sync.dma_start(out_d[c * 128:(c + 1) * 128, :], out_sb[c][:])

    if split:
        _split_sync_waits(nc)
    return nc


_NC_CACHE = None


def _get_nc():
    global _NC_CACHE
    if _NC_CACHE is None:
        _NC_CACHE = build_nc()
    return _NC_CACHE


def _prep_in_maps(h, adj, W, a):
    h = np.ascontiguousarray(h, dtype=np.float32)
    adj = np.ascontiguousarray(adj, dtype=np.int32)
    W = np.ascontiguousarray(W, dtype=np.float32)
    a = np.ascontiguousarray(a, dtype=np.float32)
    amat = np.zeros((FO, 2 * H), dtype=np.float32)
    for hh in range(H):
        amat[hh * FOH:(hh + 1) * FOH, hh] = a[hh, :FOH]
        amat[hh * FOH:(hh + 1) * FOH, H + hh] = a[hh, FOH:]
    return [
        {"h": h[c], "adj": adj[c], "W": W, "A": amat}
        for c in range(N_CORES)
    ]


def run(h, adj, W, a, trace=False, **kw):
    nc = _get_nc()
    in_maps = _prep_in_maps(h, adj, W, a)
    res = run_bass_kernel_spmd(nc, in_maps, list(range(N_CORES)), trace=trace, **kw)
    out = np.stack([res.results[c]["out"] for c in range(N_CORES)], axis=0)
    return out.astype(np.float32), res


def kernel(h, adj, W, a):
    out, _ = run(h, adj, W, a)
    return out
